# revision 18
# baseline (speedup 1.0000x reference)
"""Trainium2 Bass kernel for nn_CausalRankKAttention.

Blend of banded-softmax attention and cumsum linear attention, per (n,h) pair.
16 pairs sharded over 8 NeuronCores (2 pairs/core), no cross-core comm.

Design (v8):
  - feature map phi(x)=tanh(x)+1 on HOST; only the exp table ever loads on ACT.
  - all matmuls bf16. Microbenchmarked PE facts for this environment: column
    rate is 1 col/cycle at 1.2GHz regardless of dtype (fp8 DoubleRow included),
    the clock never ramps to 2.4GHz, LDWEIGHTS is fully hidden, and
    stationary/bank switches are free. So PE floor = total moving columns, and
    ACT (exp) is the co-dominant engine: minimize ACT columns + instruction
    count, then keep both engines decoupled.
  - per iteration n (ascending):
      band: transposed tile st[s=n, q in n..n+2) for BOTH pairs into one psum
        bank -> ONE merged exp -> per-pair masks on DVE.
      tail blocks run one-or-more iterations EARLY (blocks 13-15's units are
        spread over iters 12-14) so the ACT queue drains before the epilogue.
      linear: sg scores + kn state delta for both pairs share one psum bank.
      trailing output matmuls for block n-1 (vv stationary shared between
        softmax and linear paths; state-apply accumulates into the same psum).
  - input DMAs staged fine-grained across SP/ACT/Pool DGEs in consumption
    order; first compute needs only qkt cols 0:256.
  - outputs are RAW numerators/denominators; normalize + blend on host.
"""

import numpy as np
import ml_dtypes

import concourse.bass as bass
import concourse.bacc as bacc
import concourse.mybir as mybir
import concourse.tile as tile
from concourse import bass_utils

F32 = mybir.dt.float32
BF16 = mybir.dt.bfloat16
FP8 = mybir.dt.float8e4
AF = mybir.ActivationFunctionType
OP = mybir.AluOpType

N, L, H, E = 2, 2048, 8, 64
NB = L // 128            # 16 blocks/chunks of 128
TEMP = float(1.0 / np.sqrt(E))
EPS = 1e-6
PAIRS_PER_CORE = 2
NCORES = 8

_cached = {}

# tail work distribution: iteration -> list of (block, unit). unit1 covers
# s in [0, min(w,1024)), unit2 covers [1024, w). Last blocks pulled early so
# the ACT queue is empty by the epilogue.
TAIL_SCHED = {n: [(n + 1, 1)] + ([(n + 1, 2)] if (n + 1) * 128 > 1024 else [])
              for n in range(12)}
TAIL_SCHED[12] = [(13, 1), (13, 2), (14, 1)]
TAIL_SCHED[13] = [(14, 2), (15, 1)]
TAIL_SCHED[14] = [(15, 2)]
TAIL_SCHED[15] = []


def build_nc():
    nc = bacc.Bacc("TRN2", target_bir_lowering=False, debug=False,
                   num_devices=NCORES)
    P = PAIRS_PER_CORE
    # ---- dram tensors (per core) ----
    # qkt[p, :, 0] = kt (k^T + gate ext row), [p, :, 1] = qt (q^T + ones row)
    qkt = nc.dram_tensor("qkt", [P, 65, 4, 2, 512], BF16, kind="ExternalInput")
    sg = nc.dram_tensor("sg", [P, 64, 4, 2, 512], FP8, kind="ExternalInput")
    # vkn[:, i, 0:65] = [v*kl | kl] chunk i ; [:, i, 65:129] = phik chunk i
    vkn = nc.dram_tensor("vkn", [P, 128, NB, 129], BF16, kind="ExternalInput")
    m01d = nc.dram_tensor("m01d", [128, 256], BF16, kind="ExternalInput")
    svlv = nc.dram_tensor("svlv", [P, 65, NB, 384], BF16, kind="ExternalOutput")
    # two accumulator columns per block (tail split in halves); host sums
    tails = nc.dram_tensor("tails", [P, 128, 2 * NB], F32, kind="ExternalOutput")

    with tile.TileContext(nc) as tc:
        with (
            tc.tile_pool(name="const", bufs=1) as constp,
            tc.tile_pool(name="io", bufs=1) as iop,
            tc.tile_pool(name="acc", bufs=1) as accp,
            tc.tile_pool(name="work", bufs=3) as workp,
            tc.tile_pool(name="sp", bufs=3) as sp,
            tc.tile_pool(name="tailp", bufs=2, space="PSUM") as tailp,
            tc.tile_pool(name="bandp", bufs=1, space="PSUM") as bandp,
            tc.tile_pool(name="sknp", bufs=1, space="PSUM") as sknp,
            tc.tile_pool(name="otp", bufs=2, space="PSUM") as otp,
        ):
            m01_sb = constp.tile([128, 256], BF16, tag="m01")
            scratch = constp.tile([128, 8], F32, tag="scr0")

            # ---- ACT exp-table preload during the DMA window ----
            nc.gpsimd.memset(scratch[:], 0.0)
            nc.scalar.activation(scratch[:, 4:8], scratch[:, 0:4], AF.Exp)

            # ---- input tiles ----
            qkt_sbs, sg_sbs, vkn_sbs, accs, tacc = [], [], [], [], []
            for p in range(P):
                qkt_sb = iop.tile([65, 4, 2, 512], BF16, tag=f"qkt{p}")
                sg_sb = iop.tile([64, 4, 2, 512], FP8, tag=f"sg{p}")
                vkn_sb = iop.tile([128, NB, 129], BF16, tag=f"vkn{p}")
                qkt_sbs.append(qkt_sb)
                sg_sbs.append(sg_sb)
                vkn_sbs.append(vkn_sb)
                a = accp.tile([65, NB, 384], BF16, tag=f"acc{p}")
                t = accp.tile([128, 2 * NB], F32, tag=f"tails{p}")
                nc.gpsimd.memset(t[:], 0.0)
                accs.append(a)
                tacc.append(t)

            # ---- staged input DMAs, consumption order, 3 DGEs in parallel.
            # SP: qkt in 4 stages. Pool(SWDGE): sg in 3. ACT: vkn in 3 + m01.
            # Ordered by earliest consuming iteration so no stream starves.
            for p in range(P):
                nc.sync.dma_start(qkt_sbs[p][:, 0], qkt[p, :, 0])
            for p in range(P):
                nc.gpsimd.dma_start(sg_sbs[p][:, 0], sg[p, :, 0])
            for p in range(P):
                nc.scalar.dma_start(vkn_sbs[p][:, 0:2, :], vkn[p, :, 0:2, :])
            nc.scalar.dma_start(m01_sb[:], m01d[:])
            for p in range(P):
                nc.scalar.dma_start(vkn_sbs[p][:, 2:4, :], vkn[p, :, 2:4, :])
            for p in range(P):
                nc.sync.dma_start(qkt_sbs[p][:, 1], qkt[p, :, 1])
            for p in range(P):
                nc.gpsimd.dma_start(sg_sbs[p][:, 1], sg[p, :, 1])
            for p in range(P):
                nc.scalar.dma_start(vkn_sbs[p][:, 4:10, :], vkn[p, :, 4:10, :])
            for p in range(P):
                nc.sync.dma_start(qkt_sbs[p][:, 2], qkt[p, :, 2])
            for p in range(P):
                nc.gpsimd.dma_start(sg_sbs[p][:, 2:4], sg[p, :, 2:4])
            for p in range(P):
                nc.scalar.dma_start(vkn_sbs[p][:, 10:16, :],
                                    vkn[p, :, 10:16, :])
            for p in range(P):
                nc.sync.dma_start(qkt_sbs[p][:, 3], qkt[p, :, 3])

            def _stg(t, row, a, b):
                s = a // 512
                assert b <= 512 * (s + 1), (a, b)
                return t[:, s, row, a - 512 * s:b - 512 * s]

            def kt(p, a, b):
                return _stg(qkt_sbs[p], 0, a, b)

            def qt(p, a, b):
                return _stg(qkt_sbs[p], 1, a, b)

            def sgk(p, a, b):
                return _stg(sg_sbs[p], 0, a, b)

            def sgq(p, a, b):
                return _stg(sg_sbs[p], 1, a, b)

            def vv_ap(p, i):
                return vkn_sbs[p][:, i, 0:65]

            def sgkn_ap(p, c):
                return vkn_sbs[p][:, c, 65:129]

            s_cur = [None, None]
            prev = [None, None]
            for n in range(NB):
                qw = 256 if n < NB - 1 else 128
                c0, c1 = n * 128, (n + 1) * 128
                units = TAIL_SCHED[n]

                # -- 1. first tail unit MMs (their WAR waits are the latest
                # ACT ticks of iter n-1; issuing them first lets every later
                # PE instruction piggyback on the same waits) --
                tptiles = []
                for tb, unit in units[:1]:
                    lo = 0 if unit == 1 else 1024
                    hi = min(tb * 128, 1024) if unit == 1 else tb * 128
                    for p in range(P):
                        tp_ = tailp.tile([128, 1024], F32, tag="tp")
                        for off in range(lo, hi, 512):
                            n_ = min(512, hi - off)
                            nc.tensor.matmul(tp_[:, off - lo:off - lo + n_],
                                             qt(p, tb * 128, tb * 128 + 128),
                                             kt(p, off, off + n_),
                                             start=True, stop=True)
                        tptiles.append((tb, unit, p, tp_, hi - lo))
                # -- 2. band score MMs, both pairs into one bank --
                bandt = bandp.tile([128, 512], F32, tag="bandt")
                edge = ((c0 // 512) + 1) * 512
                for p in range(P):
                    if c0 + qw <= edge:
                        nc.tensor.matmul(bandt[:, 256 * p:256 * p + qw],
                                         kt(p, c0, c1), qt(p, c0, c0 + qw),
                                         start=True, stop=True,
                                         skip_group_check=True)
                    else:
                        w1 = edge - c0
                        nc.tensor.matmul(bandt[:, 256 * p:256 * p + w1],
                                         kt(p, c0, c1), qt(p, c0, edge),
                                         start=True, stop=True,
                                         skip_group_check=True)
                        nc.tensor.matmul(bandt[:, 256 * p + w1:256 * p + qw],
                                         kt(p, c0, c1), qt(p, edge, c0 + qw),
                                         start=True, stop=True,
                                         skip_group_check=True)
                # -- 3. sg + kn MMs, both pairs into one bank --
                sknt = sknp.tile([128, 512], F32, tag="sknt")
                for p in range(P):
                    nc.tensor.matmul(sknt[:, 128 * p:128 * (p + 1)],
                                     sgk(p, c0, c1), sgq(p, c0, c1),
                                     start=True, stop=True,
                                     skip_group_check=True)
                for p in range(P):
                    nc.tensor.matmul(sknt[0:64, 256 + 65 * p:321 + 65 * p],
                                     sgkn_ap(p, n), vv_ap(p, n),
                                     start=True, stop=True,
                                     skip_group_check=True)
                # -- 4. trailing output MMs for block n-1 --
                if prev[0] is not None:
                    pn = n - 1
                    # p1 first: its stat was produced later, so the first MM's
                    # wait covers both (minimal sem count on the PE queue)
                    for p in (1, 0):
                        pot = prev[p]["ot"]
                        nc.tensor.matmul(pot[:], vv_ap(p, pn),
                                         prev[p]["stat"][:], start=True,
                                         stop=(pn == 0), skip_group_check=True)
                    if pn > 0:
                        for p in (1, 0):
                            pot = prev[p]["ot"]
                            nc.tensor.matmul(pot[:, 256:384],
                                             prev[p]["s_before"][:],
                                             sgq(p, pn * 128, pn * 128 + 128),
                                             start=False, stop=True,
                                             skip_group_check=True)
                # -- 5. remaining tail unit MMs --
                for tb, unit in units[1:]:
                    lo = 0 if unit == 1 else 1024
                    hi = min(tb * 128, 1024) if unit == 1 else tb * 128
                    for p in range(P):
                        tp_ = tailp.tile([128, 1024], F32, tag="tp")
                        for off in range(lo, hi, 512):
                            n_ = min(512, hi - off)
                            nc.tensor.matmul(tp_[:, off - lo:off - lo + n_],
                                             qt(p, tb * 128, tb * 128 + 128),
                                             kt(p, off, off + n_),
                                             start=True, stop=True)
                        tptiles.append((tb, unit, p, tp_, hi - lo))
                # -- 6. ACT: ONE merged band exp, then tail exps --
                st_e = workp.tile([128, 512], BF16, tag="st_e")
                nc.scalar.activation(st_e[:, 0:256 + qw], bandt[:, 0:256 + qw],
                                     AF.Exp, scale=TEMP)
                for tb, unit, p, tp_, w_ in tptiles:
                    scr = workp.tile([128, 1024], BF16, tag="scr")
                    if unit == 1:
                        nc.scalar.activation(scr[:, 0:w_], tp_[:, 0:w_],
                                             AF.Exp, scale=TEMP,
                                             accum_out=tacc[p][:, tb:tb + 1])
                    else:
                        # ACT is the late-phase bottleneck: skip the 182ns
                        # READ_ACCUMULATOR and let idle DVE do the row sums
                        nc.scalar.activation(scr[:, 0:w_], tp_[:, 0:w_],
                                             AF.Exp, scale=TEMP)
                        nc.vector.reduce_sum(tacc[p][:, NB + tb:NB + tb + 1],
                                             scr[:, 0:w_],
                                             axis=mybir.AxisListType.X)
                # -- 7. DVE: masks + state update, then drain block n-1 --
                stats = []
                for p in range(P):
                    stat = workp.tile([128, 384], BF16, tag="stat")
                    nc.vector.tensor_tensor(stat[:, 0:qw],
                                            st_e[:, 256 * p:256 * p + qw],
                                            m01_sb[:, 0:qw], OP.mult)
                    if qw < 256:
                        nc.vector.memset(stat[:, 128:256], 0.0)
                    nc.vector.tensor_tensor(stat[:, 256:384],
                                            sknt[:, 128 * p:128 * (p + 1)],
                                            m01_sb[:, 0:128], OP.mult)
                    stats.append(stat)
                s_before = [s_cur[0], s_cur[1]]
                for p in range(P):
                    s_nxt = sp.tile([64, 65], BF16, tag=f"s{p}")
                    kn_ap = sknt[0:64, 256 + 65 * p:321 + 65 * p]
                    if n == 0:
                        nc.vector.tensor_copy(s_nxt[:], kn_ap)
                    else:
                        nc.vector.scalar_tensor_tensor(s_nxt[:], s_cur[p][:],
                                                       1.0, kn_ap,
                                                       OP.mult, OP.add)
                    s_cur[p] = s_nxt
                if prev[0] is not None:
                    for p in range(P):
                        nc.vector.tensor_copy(accs[p][:, n - 1, :],
                                              prev[p]["ot"][:])
                # -- 8. bookkeeping + periodic output drain --
                for p in range(P):
                    ot = otp.tile([65, 384], F32, tag="ot")
                    prev[p] = {"ot": ot, "stat": stats[p],
                               "s_before": s_before[p]}
                if n in (5, 9, 13):
                    for p in range(P):
                        nc.gpsimd.dma_start(svlv[p, :, n - 5:n - 1, :],
                                            accs[p][:, n - 5:n - 1, :])
                if n == NB - 1:
                    # all tail accums are complete (last unit ran at iter 14)
                    for p in range(P):
                        nc.sync.dma_start(tails[p], tacc[p][:])
                        nc.gpsimd.dma_start(svlv[p, :, 12:15, :],
                                            accs[p][:, 12:15, :])

            # ---- epilogue: finish last block for both pairs ----
            pn = NB - 1
            for p in range(P):
                pot = prev[p]["ot"]
                nc.tensor.matmul(pot[:], vv_ap(p, pn), prev[p]["stat"][:],
                                 start=True, stop=False, skip_group_check=True)
            for p in range(P):
                pot = prev[p]["ot"]
                nc.tensor.matmul(pot[:, 256:384], prev[p]["s_before"][:],
                                 sgq(p, pn * 128, pn * 128 + 128),
                                 start=False, stop=True, skip_group_check=True)
            # pipelined final drain: CAST p then its DMA on its own DGE
            nc.vector.tensor_copy(accs[0][:, pn, :], prev[0]["ot"][:])
            nc.scalar.dma_start(svlv[0, :, 15:NB, :], accs[0][:, 15:NB, :])
            nc.vector.tensor_copy(accs[1][:, pn, :], prev[1]["ot"][:])
            nc.sync.dma_start(svlv[1, :, 15:NB, :], accs[1][:, 15:NB, :])

    nc.compile()
    return nc


def host_prep(queries, keys, values, key_lengths_mask, blend):
    """Build per-core in_maps from full inputs."""
    q = np.ascontiguousarray(np.transpose(queries, (0, 2, 1, 3)))  # [N,H,L,E]
    k = np.ascontiguousarray(np.transpose(keys, (0, 2, 1, 3)))
    v = np.ascontiguousarray(np.transpose(values, (0, 2, 1, 3)))
    q = q.reshape(N * H, L, E).astype(np.float32)
    k = k.reshape(N * H, L, E).astype(np.float32)
    v = v.reshape(N * H, L, E).astype(np.float32)
    klm = np.asarray(key_lengths_mask, np.float32)  # [N, L]

    ii = np.arange(128)[:, None]
    cc = np.arange(256)[None, :]
    m01 = ((cc - ii >= 0) & (cc - ii <= 128)).astype(np.float32)

    in_maps = []
    for core in range(NCORES):
        qkts, sgs, vkns = [], [], []
        for p in range(PAIRS_PER_CORE):
            g = core * PAIRS_PER_CORE + p
            n = g // H
            qg, kg, vg = q[g], k[g], v[g]          # [L, E]
            kl = klm[n]                             # [L]
            i01 = (kl > 0).astype(np.float32)

            qkt_p = np.empty((65, 2, L), np.float32)
            qkt_p[0:64, 0] = kg.T
            qkt_p[64, 0] = -1e9 * (1.0 - i01)
            qkt_p[0:64, 1] = qg.T
            qkt_p[64, 1] = 1.0

            phiq = np.tanh(qg) + 1.0
            phik = np.tanh(kg) + 1.0
            sg_p = np.empty((64, 2, L), np.float32)
            sg_p[:, 0] = phik.T
            sg_p[:, 1] = phiq.T

            vv_full = np.empty((L, 65), np.float32)
            vv_full[:, 0:64] = vg * kl[:, None]
            vv_full[:, 64] = kl
            vkn_p = np.empty((128, NB, 129), np.float32)
            vkn_p[:, :, 0:65] = vv_full.reshape(NB, 128, 65).transpose(1, 0, 2)
            vkn_p[:, :, 65:129] = phik.reshape(NB, 128, 64).transpose(1, 0, 2)

            qkt_p = np.ascontiguousarray(
                qkt_p.reshape(65, 2, 4, 512).transpose(0, 2, 1, 3))
            sg_p = np.ascontiguousarray(
                sg_p.reshape(64, 2, 4, 512).transpose(0, 2, 1, 3))
            qkts.append(qkt_p.astype(ml_dtypes.bfloat16))
            sgs.append(sg_p.astype(ml_dtypes.float8_e4m3fn))
            vkns.append(vkn_p.astype(ml_dtypes.bfloat16))

        in_maps.append({
            "qkt": np.ascontiguousarray(np.stack(qkts)),
            "sg": np.ascontiguousarray(np.stack(sgs)),
            "vkn": np.ascontiguousarray(np.stack(vkns)),
            "m01d": np.ascontiguousarray(m01.astype(ml_dtypes.bfloat16)),
        })
    return in_maps


def assemble(results, blend):
    """Normalize + blend on host from raw numerators/denominators."""
    b = float(np.asarray(blend).reshape(-1)[0])
    full = np.empty((N, H, L, E), np.float32)
    for core in range(NCORES):
        r = results[core]
        svlv = np.asarray(r["svlv"], dtype=np.float32)   # [P, 65, NB, 384]
        tails = np.asarray(r["tails"])                   # [P, 128, 2*NB]
        for p in range(PAIRS_PER_CORE):
            g = core * PAIRS_PER_CORE + p
            n, h = g // H, g % H
            sv = svlv[p, :, :, 0:256]       # [65, block, 256]
            lv = svlv[p, :, :, 256:384]     # [65, chunk, 128]
            tl_sum = tails[p, :, 0:NB] + tails[p, :, NB:2 * NB]
            den = tl_sum.T + sv[64, :, 0:128]            # [NB, 128]
            num = sv[0:64, :, 0:128].copy()              # [64, NB, 128]
            num[:, 1:, :] += sv[0:64, 0:NB - 1, 128:256]
            lvn = lv[0:64]                               # [64, NB, 128]
            lvd = lv[64]                                 # [NB, 128]
            o = (b * num / den[None] +
                 (1.0 - b) * lvn / (lvd[None] + EPS))    # [64, NB, 128]
            full[n, h] = o.transpose(1, 2, 0).reshape(L, E)
    return np.ascontiguousarray(np.transpose(full, (0, 2, 1, 3)))


def kernel(queries, keys, values, key_lengths_mask, blend, _trace=False):
    if "nc" not in _cached:
        _cached["nc"] = build_nc()
    nc = _cached["nc"]
    in_maps = host_prep(queries, keys, values, key_lengths_mask, blend)
    res = bass_utils.run_bass_kernel_spmd(nc, in_maps, core_ids=list(range(NCORES)),
                                          trace=_trace)
    _cached["last_results"] = res
    return assemble(res.results, blend)


# revision 19
# speedup vs baseline: 1.0116x; 1.0116x over previous
"""Trainium2 Bass kernel for nn_CausalRankKAttention.

Blend of banded-softmax attention and cumsum linear attention, per (n,h) pair.
16 pairs sharded over 8 NeuronCores (2 pairs/core), no cross-core comm.

Design (v14, final):
  - feature map phi(x)=tanh(x)+1 on HOST; only the exp table ever loads on ACT.
  - all matmuls bf16. Microbenchmarked PE facts for this environment: column
    rate is 1 col/cycle at 1.2GHz regardless of dtype (fp8 DoubleRow included),
    the clock never ramps to 2.4GHz, LDWEIGHTS is fully hidden, and
    stationary/bank switches are free. So PE floor = total moving columns, and
    ACT (exp) is the co-dominant engine: minimize ACT columns + instruction
    count, then keep both engines decoupled.
  - per iteration n (ascending):
      band: transposed tile st[s=n, q in n..n+2) for BOTH pairs into one psum
        bank -> ONE merged exp -> per-pair masks on DVE.
      tail blocks run one-or-more iterations EARLY (blocks 13-15's units are
        spread over iters 12-14) so the ACT queue drains before the epilogue.
      linear: sg scores + kn state delta for both pairs share one psum bank.
      trailing output matmuls for block n-1 (vv stationary shared between
        softmax and linear paths; state-apply accumulates into the same psum).
  - input DMAs staged fine-grained across SP/ACT/Pool DGEs in consumption
    order; first compute needs only qkt cols 0:256.
  - outputs are RAW numerators/denominators; normalize + blend on host.
"""

import numpy as np
import ml_dtypes

import concourse.bass as bass
import concourse.bacc as bacc
import concourse.mybir as mybir
import concourse.tile as tile
from concourse import bass_utils

F32 = mybir.dt.float32
BF16 = mybir.dt.bfloat16
FP8 = mybir.dt.float8e4
AF = mybir.ActivationFunctionType
OP = mybir.AluOpType

N, L, H, E = 2, 2048, 8, 64
NB = L // 128            # 16 blocks/chunks of 128
TEMP = float(1.0 / np.sqrt(E))
EPS = 1e-6
PAIRS_PER_CORE = 2
NCORES = 8

_cached = {}

# tail work distribution: iteration -> list of (block, unit). unit1 covers
# s in [0, min(w,1024)), unit2 covers [1024, w). Last blocks pulled early so
# the ACT queue is empty by the epilogue.
TAIL_SCHED = {n: [(n + 1, 1)] + ([(n + 1, 2)] if (n + 1) * 128 > 1024 else [])
              for n in range(12)}
TAIL_SCHED[12] = [(13, 1), (13, 2), (14, 1)]
TAIL_SCHED[13] = [(14, 2), (15, 1)]
TAIL_SCHED[14] = [(15, 2)]
TAIL_SCHED[15] = []


def build_nc():
    nc = bacc.Bacc("TRN2", target_bir_lowering=False, debug=False,
                   num_devices=NCORES)
    P = PAIRS_PER_CORE
    # ---- dram tensors (per core) ----
    # qkt[p, :, 0] = kt (k^T + gate ext row), [p, :, 1] = qt (q^T + ones row)
    qkt = nc.dram_tensor("qkt", [P, 65, 4, 2, 512], BF16, kind="ExternalInput")
    sg = nc.dram_tensor("sg", [P, 64, 4, 2, 512], FP8, kind="ExternalInput")
    # vkn[:, i, 0:65] = [v*kl | kl] chunk i ; [:, i, 65:129] = phik chunk i
    vkn = nc.dram_tensor("vkn", [P, 128, NB, 129], BF16, kind="ExternalInput")
    m01d = nc.dram_tensor("m01d", [128, 256], BF16, kind="ExternalInput")
    svlv = nc.dram_tensor("svlv", [P, 65, NB, 384], BF16, kind="ExternalOutput")
    # two accumulator columns per block (tail split in halves); host sums
    tails = nc.dram_tensor("tails", [P, 128, 2 * NB], F32, kind="ExternalOutput")

    with tile.TileContext(nc) as tc:
        with (
            tc.tile_pool(name="const", bufs=1) as constp,
            tc.tile_pool(name="io", bufs=1) as iop,
            tc.tile_pool(name="acc", bufs=1) as accp,
            tc.tile_pool(name="work", bufs=3) as workp,
            tc.tile_pool(name="sp", bufs=3) as sp,
            tc.tile_pool(name="tailp", bufs=2, space="PSUM") as tailp,
            tc.tile_pool(name="bandp", bufs=1, space="PSUM") as bandp,
            tc.tile_pool(name="sknp", bufs=1, space="PSUM") as sknp,
            tc.tile_pool(name="otp", bufs=2, space="PSUM") as otp,
        ):
            m01_sb = constp.tile([128, 256], BF16, tag="m01")
            scratch = constp.tile([128, 8], F32, tag="scr0")

            # ---- ACT exp-table preload during the DMA window ----
            nc.gpsimd.memset(scratch[:], 0.0)
            nc.scalar.activation(scratch[:, 4:8], scratch[:, 0:4], AF.Exp)

            # ---- input tiles ----
            qkt_sbs, sg_sbs, vkn_sbs, accs, tacc = [], [], [], [], []
            for p in range(P):
                qkt_sb = iop.tile([65, 4, 2, 512], BF16, tag=f"qkt{p}")
                sg_sb = iop.tile([64, 4, 2, 512], FP8, tag=f"sg{p}")
                vkn_sb = iop.tile([128, NB, 129], BF16, tag=f"vkn{p}")
                qkt_sbs.append(qkt_sb)
                sg_sbs.append(sg_sb)
                vkn_sbs.append(vkn_sb)
                a = accp.tile([65, NB, 384], BF16, tag=f"acc{p}")
                t = accp.tile([128, 2 * NB], F32, tag=f"tails{p}")
                nc.gpsimd.memset(t[:], 0.0)
                accs.append(a)
                tacc.append(t)

            # ---- staged input DMAs, consumption order, 3 DGEs in parallel.
            # SP: qkt in 4 stages. Pool(SWDGE): sg in 3. ACT: vkn in 3 + m01.
            # Ordered by earliest consuming iteration so no stream starves.
            for p in range(P):
                nc.sync.dma_start(qkt_sbs[p][:, 0], qkt[p, :, 0])
            for p in range(P):
                nc.gpsimd.dma_start(sg_sbs[p][:, 0], sg[p, :, 0])
            for p in range(P):
                nc.scalar.dma_start(vkn_sbs[p][:, 0:4, :], vkn[p, :, 0:4, :])
            nc.scalar.dma_start(m01_sb[:], m01d[:])
            for p in range(P):
                nc.sync.dma_start(qkt_sbs[p][:, 1], qkt[p, :, 1])
            for p in range(P):
                nc.gpsimd.dma_start(sg_sbs[p][:, 1], sg[p, :, 1])
            for p in range(P):
                nc.scalar.dma_start(vkn_sbs[p][:, 4:10, :], vkn[p, :, 4:10, :])
            for p in range(P):
                nc.sync.dma_start(qkt_sbs[p][:, 2], qkt[p, :, 2])
            for p in range(P):
                nc.gpsimd.dma_start(sg_sbs[p][:, 2:4], sg[p, :, 2:4])
            for p in range(P):
                nc.scalar.dma_start(vkn_sbs[p][:, 10:16, :],
                                    vkn[p, :, 10:16, :])
            for p in range(P):
                nc.sync.dma_start(qkt_sbs[p][:, 3], qkt[p, :, 3])

            def _stg(t, row, a, b):
                s = a // 512
                assert b <= 512 * (s + 1), (a, b)
                return t[:, s, row, a - 512 * s:b - 512 * s]

            def kt(p, a, b):
                return _stg(qkt_sbs[p], 0, a, b)

            def qt(p, a, b):
                return _stg(qkt_sbs[p], 1, a, b)

            def sgk(p, a, b):
                return _stg(sg_sbs[p], 0, a, b)

            def sgq(p, a, b):
                return _stg(sg_sbs[p], 1, a, b)

            def vv_ap(p, i):
                return vkn_sbs[p][:, i, 0:65]

            def sgkn_ap(p, c):
                return vkn_sbs[p][:, c, 65:129]

            s_cur = [None, None]
            prev = [None, None]
            for n in range(NB):
                qw = 256 if n < NB - 1 else 128
                c0, c1 = n * 128, (n + 1) * 128
                units = TAIL_SCHED[n]

                # -- 1. first tail unit MMs (their WAR waits are the latest
                # ACT ticks of iter n-1; issuing them first lets every later
                # PE instruction piggyback on the same waits) --
                tptiles = []
                for tb, unit in units[:1]:
                    lo = 0 if unit == 1 else 1024
                    hi = min(tb * 128, 1024) if unit == 1 else tb * 128
                    for p in range(P):
                        tp_ = tailp.tile([128, 1024], F32, tag="tp")
                        for off in range(lo, hi, 512):
                            n_ = min(512, hi - off)
                            nc.tensor.matmul(tp_[:, off - lo:off - lo + n_],
                                             qt(p, tb * 128, tb * 128 + 128),
                                             kt(p, off, off + n_),
                                             start=True, stop=True)
                        tptiles.append((tb, unit, p, tp_, hi - lo))
                # -- 2. band score MMs, both pairs into one bank --
                bandt = bandp.tile([128, 512], F32, tag="bandt")
                edge = ((c0 // 512) + 1) * 512
                for p in range(P):
                    if c0 + qw <= edge:
                        nc.tensor.matmul(bandt[:, 256 * p:256 * p + qw],
                                         kt(p, c0, c1), qt(p, c0, c0 + qw),
                                         start=True, stop=True,
                                         skip_group_check=True)
                    else:
                        w1 = edge - c0
                        nc.tensor.matmul(bandt[:, 256 * p:256 * p + w1],
                                         kt(p, c0, c1), qt(p, c0, edge),
                                         start=True, stop=True,
                                         skip_group_check=True)
                        nc.tensor.matmul(bandt[:, 256 * p + w1:256 * p + qw],
                                         kt(p, c0, c1), qt(p, edge, c0 + qw),
                                         start=True, stop=True,
                                         skip_group_check=True)
                # -- 3. sg + kn MMs, both pairs into one bank --
                sknt = sknp.tile([128, 512], F32, tag="sknt")
                for p in range(P):
                    nc.tensor.matmul(sknt[:, 128 * p:128 * (p + 1)],
                                     sgk(p, c0, c1), sgq(p, c0, c1),
                                     start=True, stop=True,
                                     skip_group_check=True)
                for p in range(P):
                    nc.tensor.matmul(sknt[0:64, 256 + 65 * p:321 + 65 * p],
                                     sgkn_ap(p, n), vv_ap(p, n),
                                     start=True, stop=True,
                                     skip_group_check=True)
                # -- 4. trailing output MMs for block n-1 --
                if prev[0] is not None:
                    pn = n - 1
                    # p1 first: its stat was produced later, so the first MM's
                    # wait covers both (minimal sem count on the PE queue)
                    for p in (1, 0):
                        pot = prev[p]["ot"]
                        nc.tensor.matmul(pot[:], vv_ap(p, pn),
                                         prev[p]["stat"][:], start=True,
                                         stop=(pn == 0), skip_group_check=True)
                    if pn > 0:
                        for p in (1, 0):
                            pot = prev[p]["ot"]
                            nc.tensor.matmul(pot[:, 256:384],
                                             prev[p]["s_before"][:],
                                             sgq(p, pn * 128, pn * 128 + 128),
                                             start=False, stop=True,
                                             skip_group_check=True)
                # -- 5. remaining tail unit MMs --
                for tb, unit in units[1:]:
                    lo = 0 if unit == 1 else 1024
                    hi = min(tb * 128, 1024) if unit == 1 else tb * 128
                    for p in range(P):
                        tp_ = tailp.tile([128, 1024], F32, tag="tp")
                        for off in range(lo, hi, 512):
                            n_ = min(512, hi - off)
                            nc.tensor.matmul(tp_[:, off - lo:off - lo + n_],
                                             qt(p, tb * 128, tb * 128 + 128),
                                             kt(p, off, off + n_),
                                             start=True, stop=True)
                        tptiles.append((tb, unit, p, tp_, hi - lo))
                # -- 6. ACT: ONE merged band exp, then tail exps --
                st_e = workp.tile([128, 512], BF16, tag="st_e")
                nc.scalar.activation(st_e[:, 0:256 + qw], bandt[:, 0:256 + qw],
                                     AF.Exp, scale=TEMP)
                for tb, unit, p, tp_, w_ in tptiles:
                    acccol = tb if unit == 1 else NB + tb
                    scr = workp.tile([128, 1024], BF16, tag="scr")
                    nc.scalar.activation(scr[:, 0:w_], tp_[:, 0:w_],
                                         AF.Exp, scale=TEMP,
                                         accum_out=tacc[p][:, acccol:acccol + 1])
                # -- 7. DVE: masks + state update, then drain block n-1 --
                stats = []
                for p in range(P):
                    stat = workp.tile([128, 384], BF16, tag="stat")
                    nc.vector.tensor_tensor(stat[:, 0:qw],
                                            st_e[:, 256 * p:256 * p + qw],
                                            m01_sb[:, 0:qw], OP.mult)
                    if qw < 256:
                        nc.vector.memset(stat[:, 128:256], 0.0)
                    nc.vector.tensor_tensor(stat[:, 256:384],
                                            sknt[:, 128 * p:128 * (p + 1)],
                                            m01_sb[:, 0:128], OP.mult)
                    stats.append(stat)
                s_before = [s_cur[0], s_cur[1]]
                for p in range(P):
                    s_nxt = sp.tile([64, 65], BF16, tag=f"s{p}")
                    kn_ap = sknt[0:64, 256 + 65 * p:321 + 65 * p]
                    if n == 0:
                        nc.vector.tensor_copy(s_nxt[:], kn_ap)
                    else:
                        nc.vector.scalar_tensor_tensor(s_nxt[:], s_cur[p][:],
                                                       1.0, kn_ap,
                                                       OP.mult, OP.add)
                    s_cur[p] = s_nxt
                if prev[0] is not None:
                    for p in range(P):
                        nc.vector.tensor_copy(accs[p][:, n - 1, :],
                                              prev[p]["ot"][:])
                # -- 8. bookkeeping + periodic output drain --
                for p in range(P):
                    ot = otp.tile([65, 384], F32, tag="ot")
                    prev[p] = {"ot": ot, "stat": stats[p],
                               "s_before": s_before[p]}
                if n in (5, 9, 13):
                    for p in range(P):
                        nc.gpsimd.dma_start(svlv[p, :, n - 5:n - 1, :],
                                            accs[p][:, n - 5:n - 1, :])
                if n == NB - 1:
                    # all tail accums are complete (last unit ran at iter 14)
                    for p in range(P):
                        nc.sync.dma_start(tails[p], tacc[p][:])
                        nc.gpsimd.dma_start(svlv[p, :, 12:15, :],
                                            accs[p][:, 12:15, :])

            # ---- epilogue: finish last block for both pairs ----
            pn = NB - 1
            for p in range(P):
                pot = prev[p]["ot"]
                nc.tensor.matmul(pot[:], vv_ap(p, pn), prev[p]["stat"][:],
                                 start=True, stop=False, skip_group_check=True)
            for p in range(P):
                pot = prev[p]["ot"]
                nc.tensor.matmul(pot[:, 256:384], prev[p]["s_before"][:],
                                 sgq(p, pn * 128, pn * 128 + 128),
                                 start=False, stop=True, skip_group_check=True)
            # pipelined final drain: CAST p then its DMA on its own DGE
            nc.vector.tensor_copy(accs[0][:, pn, :], prev[0]["ot"][:])
            nc.scalar.dma_start(svlv[0, :, 15:NB, :], accs[0][:, 15:NB, :])
            nc.vector.tensor_copy(accs[1][:, pn, :], prev[1]["ot"][:])
            nc.sync.dma_start(svlv[1, :, 15:NB, :], accs[1][:, 15:NB, :])

    nc.compile()
    return nc


def host_prep(queries, keys, values, key_lengths_mask, blend):
    """Build per-core in_maps from full inputs."""
    q = np.ascontiguousarray(np.transpose(queries, (0, 2, 1, 3)))  # [N,H,L,E]
    k = np.ascontiguousarray(np.transpose(keys, (0, 2, 1, 3)))
    v = np.ascontiguousarray(np.transpose(values, (0, 2, 1, 3)))
    q = q.reshape(N * H, L, E).astype(np.float32)
    k = k.reshape(N * H, L, E).astype(np.float32)
    v = v.reshape(N * H, L, E).astype(np.float32)
    klm = np.asarray(key_lengths_mask, np.float32)  # [N, L]

    ii = np.arange(128)[:, None]
    cc = np.arange(256)[None, :]
    m01 = ((cc - ii >= 0) & (cc - ii <= 128)).astype(np.float32)

    in_maps = []
    for core in range(NCORES):
        qkts, sgs, vkns = [], [], []
        for p in range(PAIRS_PER_CORE):
            g = core * PAIRS_PER_CORE + p
            n = g // H
            qg, kg, vg = q[g], k[g], v[g]          # [L, E]
            kl = klm[n]                             # [L]
            i01 = (kl > 0).astype(np.float32)

            qkt_p = np.empty((65, 2, L), np.float32)
            qkt_p[0:64, 0] = kg.T
            qkt_p[64, 0] = -1e9 * (1.0 - i01)
            qkt_p[0:64, 1] = qg.T
            qkt_p[64, 1] = 1.0

            phiq = np.tanh(qg) + 1.0
            phik = np.tanh(kg) + 1.0
            sg_p = np.empty((64, 2, L), np.float32)
            sg_p[:, 0] = phik.T
            sg_p[:, 1] = phiq.T

            vv_full = np.empty((L, 65), np.float32)
            vv_full[:, 0:64] = vg * kl[:, None]
            vv_full[:, 64] = kl
            vkn_p = np.empty((128, NB, 129), np.float32)
            vkn_p[:, :, 0:65] = vv_full.reshape(NB, 128, 65).transpose(1, 0, 2)
            vkn_p[:, :, 65:129] = phik.reshape(NB, 128, 64).transpose(1, 0, 2)

            qkt_p = np.ascontiguousarray(
                qkt_p.reshape(65, 2, 4, 512).transpose(0, 2, 1, 3))
            sg_p = np.ascontiguousarray(
                sg_p.reshape(64, 2, 4, 512).transpose(0, 2, 1, 3))
            qkts.append(qkt_p.astype(ml_dtypes.bfloat16))
            sgs.append(sg_p.astype(ml_dtypes.float8_e4m3fn))
            vkns.append(vkn_p.astype(ml_dtypes.bfloat16))

        in_maps.append({
            "qkt": np.ascontiguousarray(np.stack(qkts)),
            "sg": np.ascontiguousarray(np.stack(sgs)),
            "vkn": np.ascontiguousarray(np.stack(vkns)),
            "m01d": np.ascontiguousarray(m01.astype(ml_dtypes.bfloat16)),
        })
    return in_maps


def assemble(results, blend):
    """Normalize + blend on host from raw numerators/denominators."""
    b = float(np.asarray(blend).reshape(-1)[0])
    full = np.empty((N, H, L, E), np.float32)
    for core in range(NCORES):
        r = results[core]
        svlv = np.asarray(r["svlv"], dtype=np.float32)   # [P, 65, NB, 384]
        tails = np.asarray(r["tails"])                   # [P, 128, 2*NB]
        for p in range(PAIRS_PER_CORE):
            g = core * PAIRS_PER_CORE + p
            n, h = g // H, g % H
            sv = svlv[p, :, :, 0:256]       # [65, block, 256]
            lv = svlv[p, :, :, 256:384]     # [65, chunk, 128]
            tl_sum = tails[p, :, 0:NB] + tails[p, :, NB:2 * NB]
            den = tl_sum.T + sv[64, :, 0:128]            # [NB, 128]
            num = sv[0:64, :, 0:128].copy()              # [64, NB, 128]
            num[:, 1:, :] += sv[0:64, 0:NB - 1, 128:256]
            lvn = lv[0:64]                               # [64, NB, 128]
            lvd = lv[64]                                 # [NB, 128]
            o = (b * num / den[None] +
                 (1.0 - b) * lvn / (lvd[None] + EPS))    # [64, NB, 128]
            full[n, h] = o.transpose(1, 2, 0).reshape(L, E)
    return np.ascontiguousarray(np.transpose(full, (0, 2, 1, 3)))


def kernel(queries, keys, values, key_lengths_mask, blend, _trace=False):
    if "nc" not in _cached:
        _cached["nc"] = build_nc()
    nc = _cached["nc"]
    in_maps = host_prep(queries, keys, values, key_lengths_mask, blend)
    res = bass_utils.run_bass_kernel_spmd(nc, in_maps, core_ids=list(range(NCORES)),
                                          trace=_trace)
    _cached["last_results"] = res
    return assemble(res.results, blend)
